# revision 12
# baseline (speedup 1.0000x reference)
"""AttentionPooling (segment_reduce) Trainium2 kernel.

att = sigmoid([input_rep, final_rep] @ W_lin.T + b_lin)
g   = att * (final_rep @ W_last.T + b_last)
out = segment_sum(g, graph_index, 16384)          # graph_index sorted

Strategy (8 NeuronCores, pure data-parallel, no collectives):
  graph_index is sorted, so a contiguous node range covers a contiguous
  graph range.  Host greedily packs whole graphs into "windows" of
  <= WIN_NODES nodes spanning <= 128 graphs; ~137 windows cover all 500k
  nodes = 8 cores x 17 windows.  Each core gets its windows as ONE
  unified byte stream (896B per partition per subtile, feature-major):
  [a0=f8(xin+s0) | a1=f8(xf0+s1) | onehot fp8 | xf0b bf16 | xf1b bf16],
  sliced on device with bitcast views -- one DMA per window (DMA count
  drives both startup doorbell serialization and the exit barrier).

Measured PE model (microbenched): every matmul streams out_cols x 1
cycle @2.4GHz regardless of dtype; fp8 DoubleRow packs K=256 into one
mm at no extra cost (=2x MACs/cycle).  So the kernel minimizes total
output columns per subtile.

b_lin is folded into the inputs (min-norm s with W_lin@s = b_lin; the
spill into the val path moves into bval = b_last - s[128:]@W_last.T).
Att fp8 weights are scaled x16 (lifts small weights out of e4m3's
subnormal floor); the ACT sigmoid descales via scale=1/16.  The shared
mm1 rhs is [16*Wlin2 | Wlast1] so its att half matches the x16 scale
while its val half stays x1.  b_last is NOT added on device: the seg
matmul rhs is [g | att], giving per-window [seg_g | seg_att]; the host
applies out = seg_g + bval * seg_att exactly (the bias contribution
through the gate is b[c] * sum_n oh*att).

Per 128-node subtile, 4 PE matmuls (psum bank [128,512]: att 0:256,
val 256:512):
    mm1: xf1b @ [16*Wlin2|Wlast1]  bf16, 0:512, start      (512c)
    mm2: DR (a0,a1) @ [16Wlin0;16Wlin1] fp8, att           (256c)
    mm3: xf0b @ Wlast0             bf16, val, stop         (256c)
    ACT: ag[256:512] = sigmoid(psum_att / 16)   -> bf16
    DVE: ag[0:256]   = att * psum_val           -> bf16
    PE : [oh].T @ ag[0:512] += seg_psum[128,512]  (lags SEGLAG)
Window end: ACT copies seg psum [128,512] -> out DMA.

PE: 1536 cycles/subtile streaming (640ns) in 4 instructions -- the
structural floor for bf16-val accuracy (att 2 passes, val 2, seg+bias
1 double-wide).  A PE warm-up burst masks the p-state ramp during the
initial DMA fill; window 1's transfer is split so its first half lands
before window 0 drains.  Rel err ~1.15e-2 (gate 2e-2).

Dead ends (measured, do not revisit without new evidence):
- DoubleRow does NOT halve per-out-col cost; it packs K=256 per mm.
- GpSimd tensor ops cost ~2.2us each (ucode) -- unusable per-subtile.
- Engine-seeded PSUM bias dies: start_tensor_calc zeroes the whole
  2KB bank lazily (ZERO_REGION_SIZE).
- fp8 val (even with data residuals) fails the 2e-2 gate; weight
  residuals need x16 weight scaling to escape e4m3 subnormals.
"""

import numpy as np
import ml_dtypes

import concourse.bacc as bacc
import concourse.tile as tile
from concourse import mybir
from concourse import bass_utils
from concourse._compat import with_exitstack

P = 128
HID = 256
WIN_SUB = 29                     # subtiles (128 nodes) per window
WIN_NODES = WIN_SUB * P          # 3712
WINDOWS_PER_CORE = 17
N_CORES = 8
NUM_GRAPHS = 16384
GMAX = P                         # graph span per window
SEGLAG = 3                       # seg MM trails the body by this many subtiles
WSCALE = 16.0                    # att fp8 weight scale
SPAN = 896                       # bytes per partition per subtile

BF16 = mybir.dt.bfloat16
F32 = mybir.dt.float32
FP8 = mybir.dt.float8e4
U8 = mybir.dt.uint8
npbf16 = ml_dtypes.bfloat16
npf8 = ml_dtypes.float8_e4m3

CHUNKS0 = [2, 10, WIN_SUB - 12]  # window-0 DMA chunking (subtiles)


def _build_windows(gi: np.ndarray, num_graphs: int):
    """Greedy windows: contiguous whole-graph ranges, graph span <= GMAX,
    node count <= WIN_NODES.  Returns list of (gbase, gcnt, nstart, ncnt)."""
    counts = np.bincount(gi, minlength=num_graphs)
    starts = np.concatenate([[0], np.cumsum(counts)])
    wins = []
    g = 0
    while g < num_graphs:
        base = g
        nodes = 0
        cnt = 0
        while g < num_graphs and cnt < GMAX and nodes + counts[g] <= WIN_NODES:
            nodes += int(counts[g])
            cnt += 1
            g += 1
        if cnt == 0:
            raise ValueError(f"graph {g} has {counts[g]} nodes > {WIN_NODES}")
        wins.append((base, cnt, int(starts[base]), nodes))
    return wins


# ----------------------------------------------------------------------------
# device kernel
# ----------------------------------------------------------------------------

@with_exitstack
def _device_kernel(ctx, tc, out_ap, ins, n_windows):
    nc = tc.nc
    xu_ap, w8_ap, wb_ap = ins

    consts = ctx.enter_context(tc.tile_pool(name="consts", bufs=1))
    xpool = ctx.enter_context(tc.tile_pool(name="x", bufs=2))
    x0pool = ctx.enter_context(tc.tile_pool(name="x0", bufs=1))
    agpool = ctx.enter_context(tc.tile_pool(name="ag", bufs=6))
    outpool = ctx.enter_context(tc.tile_pool(name="out", bufs=2))
    ps_sub = ctx.enter_context(tc.tile_pool(name="ps_sub", bufs=6, space="PSUM"))
    ps_seg = ctx.enter_context(tc.tile_pool(name="ps_seg", bufs=2, space="PSUM"))

    # consts: fp8 DR att weights [128,2,256]; bf16 [wcat1(512)|wlast0(256)]
    w8 = consts.tile([P, 2, HID], FP8)
    wb = consts.tile([P, 768], BF16)
    wcat1 = wb[:, 0:512]
    wlast0 = wb[:, 512:768]

    def load_consts():
        nc.scalar.dma_start(wb[:], wb_ap[:])
        nc.scalar.dma_start(w8[:], w8_ap[:])

    # PE warm-up: the tensor engine p-states ramp only under continuous
    # execution (full clock after ~3us).  The first ~13us are DMA-bound, so
    # run dummy matmuls on a scratch tile to enter the body at full clock.
    warm = consts.tile([P, P], BF16)
    nc.gpsimd.memset(warm[:], 0.0)

    n_sub = n_windows * WIN_SUB
    x_t = [None] * n_windows      # [(tile, subtile offset)]

    def load_window(w):
        if w == 0:
            x_t[w] = []
            c0 = 0
            for q, csub in enumerate(CHUNKS0):
                t = x0pool.tile([P, csub, SPAN], U8, tag=f"xc{q}")
                nc.sync.dma_start(
                    t[:], xu_ap[:, c0 * SPAN:(c0 + csub) * SPAN])
                x_t[w] += [(t, s - c0) for s in range(c0, c0 + csub)]
                c0 += csub
                if q == 0:
                    load_consts()
        else:
            t = xpool.tile([P, WIN_SUB, SPAN], U8, tag="xw")
            base = w * WIN_SUB
            if w == 1:
                # window 1 queues behind all of window 0's chunks; landing
                # its first half early avoids a stall at the 0->1 boundary
                for lo, hi in ((0, 12), (12, WIN_SUB)):
                    nc.sync.dma_start(
                        t[:, lo:hi], xu_ap[:, (base + lo) * SPAN:(base + hi) * SPAN])
            else:
                nc.sync.dma_start(
                    t[:], xu_ap[:, base * SPAN:(base + WIN_SUB) * SPAN])
            x_t[w] = [(t, s) for s in range(WIN_SUB)]

    seg_tiles = [None] * n_windows
    ag_tiles = {}
    DRM = mybir.MatmulPerfMode.DoubleRow

    def emit_body(w, s):
        ps = ps_sub.tile([P, 2 * HID], F32, tag="ps")
        t, ts = x_t[w][s]
        a01 = t[:, ts, 0:256].bitcast(FP8).rearrange("p (a b) -> p a b", a=2)
        xf0b = t[:, ts, 384:640].bitcast(BF16)
        xf1b = t[:, ts, 640:896].bitcast(BF16)
        nc.tensor.matmul(ps[:, 0:2 * HID], lhsT=xf1b, rhs=wcat1,
                         start=True, stop=False)
        nc.tensor.matmul(ps[:, 0:HID], lhsT=a01, rhs=w8[:],
                         start=False, stop=False, perf_mode=DRM)
        nc.tensor.matmul(ps[:, HID:2 * HID], lhsT=xf0b, rhs=wlast0,
                         start=False, stop=True)
        ag = agpool.tile([P, 2 * HID], BF16, tag="ag")
        nc.scalar.activation(ag[:, HID:2 * HID], ps[:, 0:HID],
                             mybir.ActivationFunctionType.Sigmoid,
                             scale=1.0 / WSCALE)
        nc.vector.tensor_tensor(ag[:, 0:HID], ag[:, HID:2 * HID],
                                ps[:, HID:2 * HID], op=mybir.AluOpType.mult)
        ag_tiles[(w, s)] = ag

    def emit_seg(w, s):
        if s == 0:
            seg_tiles[w] = ps_seg.tile([P, 2 * HID], F32, tag="seg", name="seg")
        seg = seg_tiles[w]
        t, ts = x_t[w][s]
        oh = t[:, ts, 256:384].bitcast(FP8)
        ag = ag_tiles.pop((w, s))
        nc.tensor.matmul(seg[:, :], lhsT=oh, rhs=ag[:],
                         start=(s == 0), stop=(s == WIN_SUB - 1))
        if s == WIN_SUB - 1:
            out_t = outpool.tile([P, 2 * HID], F32)
            nc.scalar.copy(out_t[:], seg[:, :])
            nc.sync.dma_start(out_ap[w * P:(w + 1) * P, :], out_t[:])

    wps = ps_seg.tile([P, 64], F32, tag="seg", name="warmps")
    for _ in range(60):
        nc.tensor.matmul(wps[:], lhsT=warm[:, 0:P], rhs=warm[:, 0:64],
                         start=True, stop=True)
    load_window(0)
    for t in range(n_sub):
        w, s = divmod(t, WIN_SUB)
        if s == 0 and w + 1 < n_windows:
            load_window(w + 1)
        emit_body(w, s)
        if t >= SEGLAG:
            emit_seg(*divmod(t - SEGLAG, WIN_SUB))
    for t in range(n_sub - SEGLAG, n_sub):
        emit_seg(*divmod(t, WIN_SUB))


def build_module(n_windows=WINDOWS_PER_CORE):
    nc = bacc.Bacc("TRN2", debug=False, num_devices=N_CORES)
    nn = n_windows * WIN_SUB
    ins = [
        nc.dram_tensor("xu", [P, nn * SPAN], U8, kind="ExternalInput").ap(),
        nc.dram_tensor("w8", [P, 2, HID], FP8, kind="ExternalInput").ap(),
        nc.dram_tensor("wb", [P, 768], BF16, kind="ExternalInput").ap(),
    ]
    out_ap = nc.dram_tensor("out", [n_windows * P, 2 * HID], F32,
                            kind="ExternalOutput").ap()
    with tile.TileContext(nc) as tc:
        _device_kernel(tc, out_ap, ins, n_windows)
    nc.compile()
    return nc


# ----------------------------------------------------------------------------
# host-side data prep
# ----------------------------------------------------------------------------

def _f8(a):
    return np.clip(a, -240.0, 240.0).astype(npf8)


_LAST_BVAL = None


def _prep(inputs, n_windows):
    global _LAST_BVAL
    gi = np.asarray(inputs["graph_index"]).astype(np.int64)
    x_in = np.asarray(inputs["input_rep"], dtype=np.float32)
    x_fin = np.asarray(inputs["final_rep"], dtype=np.float32)
    W_lin = np.asarray(inputs["W_lin"], dtype=np.float64)
    b_lin = np.asarray(inputs["b_lin"], dtype=np.float64)
    W_last = np.asarray(inputs["W_last"], dtype=np.float64)
    b_last = np.asarray(inputs["b_last"], dtype=np.float64)

    if np.any(np.diff(gi) < 0):
        order = np.argsort(gi, kind="stable")
        gi = gi[order]
        x_in = x_in[order]
        x_fin = x_fin[order]

    wins = _build_windows(gi, NUM_GRAPHS)
    budget = N_CORES * n_windows
    assert len(wins) <= budget, f"{len(wins)} windows > budget {budget}"
    wins = wins + [(NUM_GRAPHS, 0, len(gi), 0)] * (budget - len(wins))

    # fold b_lin into the node features: min-norm s with W_lin @ s = b_lin
    s_shift = np.linalg.lstsq(W_lin, b_lin, rcond=None)[0]      # [384]
    bval = b_last - s_shift[128:] @ W_last.T                     # [256]
    _LAST_BVAL = bval.astype(np.float32)
    s32 = s_shift.astype(np.float32)

    xin_s = x_in + s32[None, :128]
    xf0_s = x_fin[:, 0:P] + s32[None, 128:256]
    xf1_s = x_fin[:, P:2 * P] + s32[None, 256:384]
    a0 = _f8(xin_s)
    a1 = _f8(xf0_s)
    xf0_b = xf0_s.astype(npbf16)
    xf1_b = xf1_s.astype(npbf16)

    WlinT = W_lin.T                   # [384, 256] f64
    WlastT = W_last.T                 # [256, 256]
    w8 = np.zeros((P, 2, HID), npf8)
    w8[:, 0, :] = _f8(WSCALE * WlinT[0:P])
    w8[:, 1, :] = _f8(WSCALE * WlinT[P:2 * P])
    wb = np.zeros((P, 768), npbf16)
    wb[:, 0:256] = (WSCALE * WlinT[2 * P:3 * P]).astype(npbf16)
    wb[:, 256:512] = WlastT[P:2 * P].astype(npbf16)
    wb[:, 512:768] = WlastT[0:P].astype(npbf16)

    nn = n_windows * WIN_NODES
    jgrid = np.arange(P, dtype=np.int32)
    in_maps = []
    for c in range(N_CORES):
        xu = np.zeros((P, n_windows, WIN_SUB, SPAN), np.uint8)
        for j in range(n_windows):
            gb, gc, ns, ncnt = wins[c * n_windows + j]
            if ncnt > 0:
                def fm(src, dt):
                    blk = np.zeros((WIN_NODES, P), dt)
                    blk[:ncnt] = src[ns:ns + ncnt]
                    # -> [feat, subtile, node] -> bytes
                    return np.ascontiguousarray(
                        blk.reshape(WIN_SUB, P, P).transpose(2, 0, 1)
                    ).view(np.uint8).reshape(P, WIN_SUB, -1)
                xu[:, j, :, 0:128] = fm(a0, npf8)
                xu[:, j, :, 128:256] = fm(a1, npf8)
                xu[:, j, :, 384:640] = fm(xf0_b, npbf16)
                xu[:, j, :, 640:896] = fm(xf1_b, npbf16)
                # one-hot fp8: [node, subtile, graphslot]
                loc = np.full((WIN_NODES,), -1, np.int32)
                loc[:ncnt] = (gi[ns:ns + ncnt] - gb).astype(np.int32)
                a = loc.reshape(WIN_SUB, P)                      # [s, n]
                ohw = (a[:, :, None] == jgrid[None, None, :])    # [s, n, j]
                xu[:, j, :, 256:384] = np.ascontiguousarray(
                    ohw.transpose(1, 0, 2).astype(npf8)).view(np.uint8)
        in_maps.append({
            "xu": xu.reshape(P, nn // P * SPAN),
            "w8": w8, "wb": wb,
        })
    return wins, in_maps


def _assemble(wins, results, n_windows):
    out = np.zeros((NUM_GRAPHS, HID), np.float32)
    bval = _LAST_BVAL
    for c in range(N_CORES):
        res = results[c]["out"]
        for j in range(n_windows):
            gb, gc, _, _ = wins[c * n_windows + j]
            if gc == 0:
                continue
            r = res[j * P:j * P + gc]
            out[gb:gb + gc] = r[:, 0:HID] + bval[None, :] * r[:, HID:2 * HID]
    return out


# ----------------------------------------------------------------------------
# entry point
# ----------------------------------------------------------------------------

_CACHE = {}
LAST_RESULTS = None


def kernel(**inputs) -> np.ndarray:
    global LAST_RESULTS
    gi = np.asarray(inputs["graph_index"]).astype(np.int64)
    n_wins_needed = len(_build_windows(np.sort(gi), NUM_GRAPHS))
    n_windows = max(WINDOWS_PER_CORE, -(-n_wins_needed // N_CORES))
    if n_windows not in _CACHE:
        _CACHE[n_windows] = build_module(n_windows)
    nc = _CACHE[n_windows]
    wins, in_maps = _prep(inputs, n_windows)
    # a previously-wedged core can fail one run with
    # NRT_EXEC_UNIT_UNRECOVERABLE and reset itself; retry once
    try:
        res = bass_utils.run_bass_kernel_spmd(
            nc, in_maps, core_ids=list(range(N_CORES)))
    except Exception:
        res = bass_utils.run_bass_kernel_spmd(
            nc, in_maps, core_ids=list(range(N_CORES)))
    LAST_RESULTS = res
    return _assemble(wins, res.results, n_windows)
